# revision 7
# baseline (speedup 1.0000x reference)
"""Causal self-attention (B=4, T=2048, C=1024, H=16) on 8 Trainium2 cores.

Sharding: batch x head-half. Core c handles batch b=c//2 and heads
[8*(c%2), 8*(c%2)+8). Each core computes qkv for its head slice, causal
flash-style attention fully in SBUF, and a partial c_proj; a pairwise
ReduceScatter (cores 2b, 2b+1) sums the two head-halves and leaves each
core with 1024 rows of the final output.

v2 (all-bf16, overlap-tuned):
  - weights resident in SBUF across reps (w3r/wpr const pools); x cast to
    bf16 on host, DMA'd per rep into a double-buffered xT tile
  - causal mask applied INSIDE the S psum accumulation via an extra
    matmul (ident^T @ (-30000*strict_upper)) so exp(masked)=0 exactly --
    no DVE mask pass, and PV reads the exp output directly
  - exp runs on 1024-wide (2 psum banks) tiles: half the ACT instructions
  - softmax denominators: 65th vaug column of ones accumulates into pv;
    batched reciprocal, y-normalize on the (otherwise idle) Pool engine
  - y/yT bf16; transposes packed 4-per-psum-bank, single copy per group
  - emission interleaves attention (ACT-heavy) with qkv/v/proj chunks
    (PE-only) so both engines stay saturated; psum pools double-buffered
"""
import os
import time
from collections import OrderedDict, deque
from contextlib import ExitStack

import numpy as np
import ml_dtypes

import concourse.bass as bass
import concourse.mybir as mybir
import concourse.tile as tile
from concourse.masks import make_identity

B, T, C = 4, 2048, 1024
H, HD = 16, 64
NCORES = 8
P = 128
KC = C // P  # 8 contraction chunks
HPC = H // 2  # heads per core
HCOLS = HPC * HD  # 512 qkv columns per core
TOKTILES = T // P  # 16
F32 = mybir.dt.float32
BF16 = mybir.dt.bfloat16
NEG = -30000.0


def legalize_waits(nc):
    """This walrus build rejects >1 sem wait per instruction (>2 for
    EventSemaphore): split extras onto preceding same-engine NOPs."""
    for f in nc.m.functions:
        for bb in f.blocks:
            new_insts = []
            for inst in bb.instructions:
                si = inst.sync_info
                cap = 2 if isinstance(inst, mybir.InstEventSemaphore) else 1
                if si is not None and si.on_wait and len(si.on_wait) > cap:
                    waits = list(si.on_wait)
                    extra, keep = waits[:-cap], waits[-cap:]
                    for k, w in enumerate(extra):
                        new_insts.append(
                            mybir.InstNoOp(
                                name=f"{inst.name}-splitw{k}",
                                engine=inst.engine,
                                sync_info=mybir.SyncInfo(on_wait=[w], on_update=[]),
                            )
                        )
                    si.on_wait = keep
                    inst.sync_info = si
                new_insts.append(inst)
            bb.instructions = new_insts


def build_nc(reps: int = 1, rs_bf16: bool = True, no_rs: bool = False):
    nc = bass.Bass()
    xt_in = nc.declare_dram_parameter("xt", [C, T], BF16, isOutput=False)
    w3_in = nc.declare_dram_parameter("w3", [C, 3 * HCOLS], BF16, isOutput=False)
    wp_in = nc.declare_dram_parameter("wp", [HCOLS, C], BF16, isOutput=False)
    bq_in = nc.declare_dram_parameter("bq", [4, P, 1], F32, isOutput=False)
    bk_in = nc.declare_dram_parameter("bk", [4, P, 1], F32, isOutput=False)
    bvb_in = nc.declare_dram_parameter("bvb", [P, HCOLS], F32, isOutput=False)
    bpb_in = nc.declare_dram_parameter("bpb", [P, C], F32, isOutput=False)
    masks_in = nc.declare_dram_parameter("masks", [1, P, P], BF16, isOutput=False)
    rs_dt = BF16 if rs_bf16 else F32
    out_p = nc.declare_dram_parameter("out_part", [T // 2, C], rs_dt, isOutput=True)

    with tile.TileContext(nc) as tc, ExitStack() as top:
        dram = top.enter_context(tc.tile_pool(name="dram", bufs=1, space="DRAM"))
        partial = [
            dram.tile([512, 512], rs_dt, tag=f"partial{g}", name=f"partial{g}")
            for g in range(8)
        ]
        rs_out = [
            dram.tile([256, 512], rs_dt, tag=f"rs{g}", name=f"rs{g}") for g in range(8)
        ]

        const = top.enter_context(tc.tile_pool(name="const", bufs=1))
        # weights live in SBUF across all reps
        w3r = const.tile([P, KC, 3 * HCOLS], BF16, name="w3r")
        for cc in range(4):
            sl = slice(cc * 384, (cc + 1) * 384)
            nc.sync.dma_start(
                w3r[:, :, sl],
                w3_in[:, sl].rearrange("(kc p) m -> p kc m", p=P),
            )
        wpr = const.tile([P, 4, C], BF16, name="wpr")
        nc.sync.dma_start(wpr[:], wp_in[:].rearrange("(kc p) m -> p kc m", p=P))
        maskneg = const.tile([P, P], BF16, name="maskneg")
        nc.sync.dma_start(maskneg[:], masks_in[0])
        bq_t = [const.tile([P, 1], F32, tag=f"bq{m}", name=f"bq{m}") for m in range(4)]
        bk_t = [const.tile([P, 1], F32, tag=f"bk{m}", name=f"bk{m}") for m in range(4)]
        for m in range(4):
            nc.sync.dma_start(bq_t[m][:], bq_in[m])
            nc.sync.dma_start(bk_t[m][:], bk_in[m])
        bvb = const.tile([P, HCOLS], F32, name="bvb")
        nc.sync.dma_start(bvb[:], bvb_in[:])
        bpb = const.tile([P, C], F32, name="bpb")
        nc.sync.dma_start(bpb[:], bpb_in[:])
        ident = const.tile([P, P], BF16, name="ident")
        make_identity(nc, ident[:])

        def body():
            with ExitStack() as ctx:
                xt_pool = ctx.enter_context(tc.tile_pool(name="xt", bufs=2))
                xT = xt_pool.tile([P, KC, T], BF16, tag="xT", name="xT")
                for kk in range(4):
                    for tck in range(4):
                        nc.sync.dma_start(
                            xT[:, 2 * kk : 2 * kk + 2, tck * 512 : (tck + 1) * 512],
                            xt_in[
                                kk * 256 : (kk + 1) * 256, tck * 512 : (tck + 1) * 512
                            ].rearrange("(kc p) t -> p kc t", p=P),
                        )

                sb_pool = ctx.enter_context(tc.tile_pool(name="sb", bufs=1))
                qT = sb_pool.tile([P, 4, T], BF16, tag="qT", name="qT")
                kT = sb_pool.tile([P, 4, T], BF16, tag="kT", name="kT")
                vaug = sb_pool.tile([P, TOKTILES, HPC, HD + 1], BF16, tag="vaug", name="vaug")
                y = sb_pool.tile([P, TOKTILES, HCOLS], BF16, tag="y", name="y")
                yT = sb_pool.tile([P, 4, T], BF16, tag="yT", name="yT")
                rec_pool = ctx.enter_context(tc.tile_pool(name="rec", bufs=4))
                att_pool = ctx.enter_context(tc.tile_pool(name="att", bufs=4))
                ob_pool = ctx.enter_context(tc.tile_pool(name="ob", bufs=3))
                # psum banks: sp 2x2 + pv 1 + qkps 2 + tb 1 = 8
                s_ps = ctx.enter_context(tc.tile_pool(name="s_ps", bufs=2, space="PSUM"))
                pv_ps = ctx.enter_context(tc.tile_pool(name="pv_ps", bufs=1, space="PSUM"))
                qk_ps = ctx.enter_context(tc.tile_pool(name="qk_ps", bufs=2, space="PSUM"))

                # ---- PE-only filler chunks ----
                def v_chunk(t):
                    vp = qk_ps.tile([P, HCOLS], F32, tag="qkps", name="vp")
                    for k in range(KC):
                        nc.tensor.matmul(
                            vp[:],
                            xT[:, k, t * P : (t + 1) * P],
                            w3r[:, k, 2 * HCOLS : 3 * HCOLS],
                            start=(k == 0),
                            stop=(k == KC - 1),
                        )
                    nc.vector.memset(vaug[:, t, :, HD : HD + 1], 1.0)
                    nc.vector.tensor_add(
                        vaug[:, t, :, 0:HD],
                        vp[:].rearrange("p (h d) -> p h d", d=HD),
                        bvb[:].rearrange("p (h d) -> p h d", d=HD),
                    )

                def qk_chunk(m, part, tck):
                    qp = qk_ps.tile([P, HCOLS], F32, tag="qkps", name="qp")
                    col0 = part * HCOLS + m * P
                    for k in range(KC):
                        nc.tensor.matmul(
                            qp[:],
                            w3r[:, k, col0 : col0 + P],
                            xT[:, k, tck * 512 : (tck + 1) * 512],
                            start=(k == 0),
                            stop=(k == KC - 1),
                        )
                    dst, bias = (qT, bq_t) if part == 0 else (kT, bk_t)
                    nc.vector.tensor_scalar_add(
                        dst[:, m, tck * 512 : (tck + 1) * 512], qp[:], bias[m][:]
                    )

                def transp_chunk(m, qc):
                    tb = qk_ps.tile([P, 512], BF16, tag="tb", bufs=1, name="tb")
                    for i in range(4):
                        t = qc * 4 + i
                        nc.tensor.transpose(
                            tb[:, i * P : (i + 1) * P],
                            y[:, t, m * P : (m + 1) * P],
                            ident[:],
                        )
                    nc.vector.tensor_copy(yT[:, m, qc * 512 : (qc + 1) * 512], tb[:])

                def proj_chunk(qc, ncol, sb):
                    t = qc * 4 + sb
                    pt = qk_ps.tile([P, HCOLS], F32, tag="qkps", name="pt")
                    for k in range(4):
                        nc.tensor.matmul(
                            pt[:],
                            yT[:, k, t * P : (t + 1) * P],
                            wpr[:, k, ncol * 512 : (ncol + 1) * 512],
                            start=(k == 0),
                            stop=(k == 3),
                        )
                    ob = ob_pool.tile([P, HCOLS], rs_dt, tag="ob", name="ob")
                    nc.vector.tensor_add(
                        ob[:], pt[:], bpb[:, ncol * 512 : (ncol + 1) * 512]
                    )
                    nc.sync.dma_start(
                        partial[2 * qc + ncol][sb * P : (sb + 1) * P, :], ob[:]
                    )

                def rs_chunk(qc, ncol):
                    g = 2 * qc + ncol
                    if not no_rs:
                        nc.gpsimd.collective_compute(
                            "ReduceScatter",
                            mybir.AluOpType.add,
                            replica_groups=[[0, 1], [2, 3], [4, 5], [6, 7]],
                            ins=[partial[g].opt()],
                            outs=[rs_out[g].opt()],
                        )
                    nc.sync.dma_start(
                        out_p[qc * 256 : (qc + 1) * 256, ncol * 512 : (ncol + 1) * 512],
                        rs_out[g][:],
                    )

                # ---- attention for head (2m+hh), q-chunk qc ----
                def attention(m, hh, qc):
                    hpc = 2 * m + hh
                    hsl = slice(hh * HD, (hh + 1) * HD)
                    nk = 4 * qc + 4
                    npairs = nk // 2
                    pv = pv_ps.tile([P, 4, HD + 1], F32, tag="pv", name="pv")
                    ats = [None] * npairs

                    def emit_pv(j):
                        at = ats[j]
                        for half in (0, 1):
                            kt = 2 * j + half
                            for sb in range(4):
                                if kt > qc * 4 + sb:
                                    continue
                                # start=True clears has_written bits for the
                                # WHOLE bank, so only the bank's first matmul
                                # may set it; the other groups' first writes
                                # land on cleared bits and overwrite anyway.
                                nc.tensor.matmul(
                                    pv[:, sb, :],
                                    at[:, half * 512 + sb * P : half * 512 + (sb + 1) * P],
                                    vaug[:, kt, hpc, :],
                                    start=(kt == 0 and sb == 0),
                                    stop=(kt == qc * 4 + sb),
                                    skip_group_check=True,
                                )

                    for j in range(npairs):
                        sp = s_ps.tile([P, 1024], F32, tag="sp", name="sp")
                        o0 = max(0, 2 * j * P - qc * 512)
                        for half in (0, 1):
                            kt = 2 * j + half
                            d = max(0, kt * P - qc * 512)
                            diag = kt * P - qc * 512 >= 0
                            nc.tensor.matmul(
                                sp[:, half * 512 + d : half * 512 + 512],
                                kT[hsl, m, kt * P : (kt + 1) * P],
                                qT[hsl, m, qc * 512 + d : (qc + 1) * 512],
                                start=True,
                                stop=not diag,
                            )
                            if diag:
                                nc.tensor.matmul(
                                    sp[:, half * 512 + d : half * 512 + d + P],
                                    ident[:],
                                    maskneg[:],
                                    start=False,
                                    stop=True,
                                )
                        at = att_pool.tile([P, 1024], BF16, tag="at", name="at")
                        nc.scalar.activation(
                            at[:, o0:1024],
                            sp[:, o0:1024],
                            mybir.ActivationFunctionType.Exp,
                            scale=0.125,
                        )
                        ats[j] = at
                        if j >= 1:
                            emit_pv(j - 1)
                    emit_pv(npairs - 1)

                    rec = rec_pool.tile([P, 4, 1], F32, tag="rec", name="rec")
                    nc.vector.reciprocal(rec[:], pv[:, :, HD : HD + 1])
                    for sb in range(4):
                        t = qc * 4 + sb
                        nc.vector.tensor_scalar_mul(
                            y[:, t, hpc * HD : (hpc + 1) * HD],
                            pv[:, sb, 0:HD],
                            rec[:, sb, :],
                        )

                # ---- emission scheduling: attention + PE fillers ----
                PE_NS = 0.4167
                ACT_NS = 0.8333

                fillers = OrderedDict()  # key -> list of (est_ns, closure)
                for qcn in range(4):
                    for m in range(4):
                        fillers[("qk", m, qcn)] = [
                            (1900, (lambda m=m, p=p, t=qcn: qk_chunk(m, p, t)))
                            for p in (0, 1)
                        ]
                for t in range(TOKTILES):
                    fillers[("v", t)] = [(1900, (lambda t=t: v_chunk(t)))]

                def run_key(key):
                    for _, fn in fillers.pop(key, ()):
                        fn()

                def pop_fillers(ns):
                    while ns > 0 and fillers:
                        key = next(iter(fillers))
                        lst = fillers[key]
                        est, fn = lst.pop(0)
                        if not lst:
                            del fillers[key]
                        fn()
                        ns -= est
                    return ns

                def est_call(qc):
                    cols = 2048 * qc + 1280
                    act = cols * ACT_NS + (2 * qc + 2) * 260
                    pe = cols * PE_NS + 4 * 53 + (16 * qc + 10) * 40
                    return act - pe

                # prologue: only what the first attention call needs
                run_key(("qk", 0, 0))
                run_key(("v", 0)), run_key(("v", 1))
                run_key(("v", 2)), run_key(("v", 3))

                for qc in range(4):
                    for t in range(4 * qc, 4 * qc + 4):
                        run_key(("v", t))
                    for m in range(4):
                        run_key(("qk", m, qc))
                        for hh in (0, 1):
                            attention(m, hh, qc)
                            pop_fillers(est_call(qc))
                    # proj for this qc becomes filler work for qc+1
                    pj = []
                    for m in range(4):
                        pj.append((450, (lambda m=m, qc=qc: transp_chunk(m, qc))))
                    for ncol in range(2):
                        for sb in range(4):
                            pj.append(
                                (1050, (lambda qc=qc, n=ncol, s=sb: proj_chunk(qc, n, s)))
                            )
                        pj.append((400, (lambda qc=qc, n=ncol: rs_chunk(qc, n))))
                    fillers[("proj", qc)] = pj

                while fillers:
                    pop_fillers(1 << 30)

        for _ in range(reps):
            body()

    legalize_waits(nc)
    return nc


def prep_inputs(x, W_qkv, b_qkv, W_proj, b_proj):
    x = np.asarray(x, dtype=np.float32)
    W_qkv = np.asarray(W_qkv, dtype=np.float32)
    b_qkv = np.asarray(b_qkv, dtype=np.float32)
    W_proj = np.asarray(W_proj, dtype=np.float32)
    b_proj = np.asarray(b_proj, dtype=np.float32)
    BF = ml_dtypes.bfloat16

    xTs = [np.ascontiguousarray(x[b].T).astype(BF) for b in range(B)]
    halves = []
    for half in range(2):
        c0 = half * HCOLS
        w3 = np.ascontiguousarray(
            np.concatenate(
                [
                    W_qkv[:, c0 : c0 + HCOLS],
                    W_qkv[:, C + c0 : C + c0 + HCOLS],
                    W_qkv[:, 2 * C + c0 : 2 * C + c0 + HCOLS],
                ],
                axis=1,
            )
        ).astype(BF)
        wp = np.ascontiguousarray(W_proj[c0 : c0 + HCOLS, :]).astype(BF)
        bq = np.ascontiguousarray(b_qkv[c0 : c0 + HCOLS].reshape(4, P, 1))
        bk = np.ascontiguousarray(b_qkv[C + c0 : C + c0 + HCOLS].reshape(4, P, 1))
        bvb = np.tile(b_qkv[2 * C + c0 : 2 * C + c0 + HCOLS], (P, 1))
        halves.append((w3, wp, bq, bk, np.ascontiguousarray(bvb)))
    # both cores of a pair add the proj bias before the ReduceScatter
    # sums them, so each adds half
    bpb = np.ascontiguousarray(np.tile(b_proj / 2.0, (P, 1)))

    kk = np.arange(P)[:, None]
    qq = np.arange(P)[None, :]
    masks = np.where(kk > qq, NEG, 0.0).astype(BF)[None]

    in_maps = []
    for c in range(NCORES):
        b, half = c // 2, c % 2
        w3, wp, bq, bk, bvb = halves[half]
        in_maps.append(
            {
                "xt": xTs[b],
                "w3": w3,
                "wp": wp,
                "bq": bq,
                "bk": bk,
                "bvb": bvb,
                "bpb": bpb,
                "masks": masks,
            }
        )
    return in_maps


class _Runner:
    """Build-once SPMD executor via PJRT (mirrors bass2jax.run_bass_via_pjrt)."""

    def __init__(self, nc, n_cores=NCORES):
        import jax
        from jax.sharding import Mesh, PartitionSpec, NamedSharding
        from jax.experimental.shard_map import shard_map
        from concourse.bass2jax import (
            _bass_exec_p,
            install_neuronx_cc_hook,
            partition_id_tensor,
        )

        self.jax = jax
        install_neuronx_cc_hook()
        partition_name = (
            nc.partition_id_tensor.name if nc.partition_id_tensor else None
        )
        in_names, out_names, out_avals, zero_outs = [], [], [], []
        for alloc in nc.m.functions[0].allocations:
            if not isinstance(alloc, mybir.MemoryLocationSet):
                continue
            name = alloc.memorylocations[0].name
            if alloc.kind == "ExternalInput":
                if name != partition_name:
                    in_names.append(name)
            elif alloc.kind == "ExternalOutput":
                shape = tuple(alloc.tensor_shape)
                dtype = mybir.dt.np(alloc.dtype)
                out_names.append(name)
                out_avals.append(jax.core.ShapedArray(shape, dtype))
                zero_outs.append(np.zeros(shape, dtype))
        self.in_names, self.out_names = in_names, out_names
        self.out_avals, self.zero_outs = out_avals, zero_outs
        self.n_cores = n_cores
        n_params = len(in_names)
        self.n_params = n_params
        all_in = list(in_names) + list(out_names)
        if partition_name is not None:
            all_in.append(partition_name)
        donate = tuple(range(n_params, n_params + len(out_names)))

        def _body(*args):
            operands = list(args)
            if partition_name is not None:
                operands.append(partition_id_tensor())
            outs = _bass_exec_p.bind(
                *operands,
                out_avals=tuple(out_avals),
                in_names=tuple(all_in),
                out_names=tuple(out_names),
                lowering_input_output_aliases=(),
                sim_require_finite=True,
                sim_require_nnan=True,
                nc=nc,
            )
            return tuple(outs)

        devices = jax.devices()[:n_cores]
        self.mesh = Mesh(np.asarray(devices), ("core",))
        in_specs = (PartitionSpec("core"),) * (n_params + len(out_names))
        out_specs = (PartitionSpec("core"),) * len(out_names)
        self.sharding = NamedSharding(self.mesh, PartitionSpec("core"))
        self.jitted = jax.jit(
            shard_map(
                _body,
                mesh=self.mesh,
                in_specs=in_specs,
                out_specs=out_specs,
                check_rep=False,
            ),
            donate_argnums=donate,
            keep_unused=True,
        )

    def put_inputs(self, in_maps):
        per_core = [[np.asarray(m[n]) for n in self.in_names] for m in in_maps]
        concat = [
            np.concatenate([per_core[c][i] for c in range(self.n_cores)], axis=0)
            for i in range(self.n_params)
        ]
        return [self.jax.device_put(a, self.sharding) for a in concat]

    def _zeros(self):
        return [
            self.jax.device_put(
                np.zeros((self.n_cores * z.shape[0], *z.shape[1:]), z.dtype),
                self.sharding,
            )
            for z in self.zero_outs
        ]

    def run(self, dev_inputs, n_timed=0):
        out = self.jitted(*dev_inputs, *self._zeros())
        self.jax.block_until_ready(out)
        times = []
        for _ in range(n_timed):
            z = self._zeros()
            self.jax.block_until_ready(z)
            t0 = time.perf_counter()
            out2 = self.jitted(*dev_inputs, *z)
            self.jax.block_until_ready(out2)
            times.append(time.perf_counter() - t0)
            out = out2
        np_outs = [np.asarray(a) for a in out]
        results = [
            {
                n: np_outs[i].reshape(self.n_cores, *self.out_avals[i].shape)[c]
                for i, n in enumerate(self.out_names)
            }
            for c in range(self.n_cores)
        ]
        return results, times


_RUNNERS = {}


def get_runner(reps: int = 1, **kw) -> _Runner:
    key = (reps, tuple(sorted(kw.items())))
    if key not in _RUNNERS:
        _RUNNERS[key] = _Runner(build_nc(reps, **kw))
    return _RUNNERS[key]


def kernel(x, W_qkv, b_qkv, W_proj, b_proj):
    in_maps = prep_inputs(x, W_qkv, b_qkv, W_proj, b_proj)
    runner = get_runner(1)
    results, _ = runner.run(runner.put_inputs(in_maps))
    out = np.empty((B, T, C), dtype=np.float32)
    for c in range(NCORES):
        b, rank = c // 2, c % 2
        part = results[c]["out_part"]
        for g in range(4):
            r0 = g * 512 + rank * 256
            out[b, r0 : r0 + 256, :] = np.asarray(
                part[g * 256 : (g + 1) * 256, :], dtype=np.float32
            )
    return out
